# revision 6
# baseline (speedup 1.0000x reference)
"""Trainium2 Bass kernel for DiscreteRotation (moe_routing).

Per sample: k = argmax(mean_hw(x) @ W + b); out = rot90(x, k, axes=(H,W)).

Pure data parallel over 8 NeuronCores (8 samples each). The rotation is a
pure permutation of the data, and the correctness gate is rel_err < 2e-2,
so the device pipeline runs in bfloat16: the host rounds x to bf16 (max
elementwise rel err 2^-9 ~ 0.2%), the device moves/permutes bf16 rows, and
the host upcasts the result. This halves HBM traffic per direction, which
is the roofline for this memory-bound problem (51.4 MB/core/direction in
f32 -> 25.7 MB in bf16 at 360 GB/s aggregate DMA).

Runtime branching is not supported by this execution path, so the routing
is SPECULATED: the classifier bias dominates the near-zero image means of
this input regime, so k_prior = argmax(b) predicts every sample's route. A
single static HW launch rotates every sample per the speculated pattern AND
computes per-sample per-channel pixel sums from the same SBUF-resident bf16
data (halving adds into a scratch tile + strided f32 reduce on DVE,
partition-collapse ones-matmul on PE after the transposes so it never
stalls them); the host turns the sums into logits/argmax and fixes up any
mispredicted sample with numpy rot90 on the original f32 data (correct for
arbitrary inputs, never triggered by the bias-dominated regime).

Queue discipline (each HWDGE queue drains in program order, so mixing
producers stalls the DMA bus): loads ride SP, stores ride Activation, the
identity load rides Pool/SWDGE, and the single channel-sums DMA fires once
at the end. The last sample's w=1 copies/stores are split by partition
halves to shorten the exposed pipeline drain.

Per-slot rotation code (all fully static):
  r=0: load -> store.
  r=1: load -> pixel-reverse within rows (DVE/ACT, negative-stride input
       APs) -> pixel-transpose on PE (per-channel [<=128 x <=128] tiles via
       identity matmul, PSUM -> DVE/ACT copy) -> store.
  r=2: rot90 applied twice.
  r=3: pixel-transpose, copy-out at reversed pixel positions, each output
       half stored as soon as its copies land -> store.

Every DMA descriptor is a contiguous image (half-)row on both HBM and SBUF
sides (>= 6 KB in bf16), so all transfers run at full HBM rate. SBUF row
layout: image row r lives at partition r%128, slot r//128 (slot0: rows
0..127, slot1: rows 128..223).
"""
import numpy as np
import ml_dtypes
from contextlib import ExitStack

import concourse.bass as bass
import concourse.bacc as bacc
import concourse.tile as tile
import concourse.mybir as mybir
from concourse.bass_utils import run_bass_kernel_spmd

F32 = mybir.dt.float32
BF16 = mybir.dt.bfloat16
NP_BF16 = ml_dtypes.bfloat16

N_CORES = 8
H = 224
W = 224
C = 32
ROW = W * C          # 7168 elems per image row (14336 B in bf16)
P0, P1 = 128, 96     # rows in slot 0 / slot 1
GRP = 8


def _flip(ap: bass.AP, dim: int) -> bass.AP:
    """Reverse iteration order of one AP dim."""
    pairs = [list(p) for p in ap.ap]
    stride, num = pairs[dim]
    off = ap.offset + stride * (num - 1)
    pairs[dim] = [-stride, num]
    return bass.AP(ap.tensor, off, pairs)


def _pixview(ap2d: bass.AP) -> bass.AP:
    """[p, ROW-range] -> [p, c(32), j(224)] channel-major pixel view."""
    return ap2d.rearrange("p (j c) -> p c j", j=W, c=C)


# load quarters: (slot, window, nrows, npix); window-major so the w=0
# transposes start as soon as their data lands
QUARTS = [(0, 0, P0, P0), (1, 0, P1, P0), (0, 1, P0, P1), (1, 1, P1, P1)]


def _build_rotate(S: int, pattern: tuple, with_cls: bool = True) -> bacc.Bacc:
    """Static program: sample slot s gets rotation pattern[s].

    with_cls: additionally compute per-sample per-channel pixel sums from
    the loaded data and write them to "cs" [S, C] — lets the host verify a
    speculated pattern without a separate pass over x.
    """
    nc = bacc.Bacc("TRN2", target_bir_lowering=False, debug=False,
                   num_devices=N_CORES)
    x = nc.dram_tensor("x", [S * H, ROW], BF16, kind="ExternalInput").ap()
    ident = nc.dram_tensor("ident", [128, 128], BF16,
                           kind="ExternalInput").ap()
    y = nc.dram_tensor("y", [S * H, ROW], BF16, kind="ExternalOutput").ap()
    if with_cls:
        cs = nc.dram_tensor("cs", [S, C], F32, kind="ExternalOutput").ap()

    with tile.TileContext(nc) as tc:
        with ExitStack() as ctx:
            cpool = ctx.enter_context(tc.tile_pool(name="consts", bufs=1))
            apool = ctx.enter_context(tc.tile_pool(name="A", bufs=2))
            bpool = ctx.enter_context(tc.tile_pool(name="B", bufs=2))
            tpool = ctx.enter_context(
                tc.tile_pool(name="ptrans", bufs=4, space="PSUM"))

            it = cpool.tile([128, 128], BF16)
            # Pool/SWDGE queue: keeps SP free for the first x loads
            nc.gpsimd.dma_start(out=it[:], in_=ident)
            if with_cls:
                spool = ctx.enter_context(tc.tile_pool(name="small", bufs=2))
                mpool = ctx.enter_context(
                    tc.tile_pool(name="pmean", bufs=2, space="PSUM"))
                ones = cpool.tile([128, 1], F32)
                pmall = cpool.tile([1, S * C], F32)
                nc.vector.memset(ones[:], 1.0)

            def classify_sums(s, A):
                """Per-channel pixel sums: halving adds (bf16, DVE 2x mode)
                into a scratch tile (A stays intact for the rotation), then
                a small strided f32 reduce. Returns the per-row sums tile;
                the partition collapse happens on PE after the transposes."""
                sc = spool.tile([128, 2 * 112 * C], BF16, name=f"sc{s}",
                                tag="sc")
                sums = spool.tile([128, 2 * C], F32, name=f"sm{s}", tag="sm")
                with nc.allow_low_precision(reason="bf16 mean-pool; bias "
                                            "gaps dwarf rounding noise"):
                    for sl, n in ((0, P0), (1, P1)):
                        base = sl * ROW
                        half = 112 * C
                        # first halving writes scratch, rest halve in place
                        nc.vector.tensor_add(
                            out=sc[0:n, sl * 112 * C:sl * 112 * C + half],
                            in0=A[0:n, base:base + half],
                            in1=A[0:n, base + half:base + 2 * half])
                        npix = 112
                        sbase = sl * 112 * C
                        while npix % 2 == 0 and npix > 14:
                            h2 = npix // 2 * C
                            nc.vector.tensor_add(
                                out=sc[0:n, sbase:sbase + h2],
                                in0=sc[0:n, sbase:sbase + h2],
                                in1=sc[0:n, sbase + h2:sbase + 2 * h2])
                            npix //= 2
                        av = sc[0:n, sbase:sbase + npix * C]
                        nc.vector.tensor_reduce(
                            out=sums[0:n, sl * C:(sl + 1) * C],
                            in_=bass.AP(av.tensor, av.offset,
                                        [list(av.ap[0]), [1, C], [C, npix]]),
                            axis=mybir.AxisListType.X, op=mybir.AluOpType.add)
                return sums

            def classify_collapse(s, sums):
                """Partition-collapse the per-row sums (PE, post-transposes)
                and park the result in the persistent pmall tile."""
                pm = mpool.tile([1, C], F32, name=f"pm{s}", tag="pm")
                nc.tensor.matmul(pm[0:1, 0:C], lhsT=ones[0:P0, 0:1],
                                 rhs=sums[0:P0, 0:C], start=True, stop=False)
                nc.tensor.matmul(pm[0:1, 0:C], lhsT=ones[0:P1, 0:1],
                                 rhs=sums[0:P1, C:2 * C], start=False,
                                 stop=True)
                nc.vector.tensor_copy(out=pmall[0:1, s * C:(s + 1) * C],
                                      in_=pm[0:1, 0:C])

            def load(s, A):
                for sl, w_, n, npx in QUARTS:
                    off = sl * ROW + w_ * 128 * C
                    xr = x[s * H + sl * 128:s * H + sl * 128 + n,
                           w_ * 128 * C:w_ * 128 * C + npx * C]
                    nc.sync.dma_start(out=A[0:n, off:off + npx * C], in_=xr)

            def store_fwd(s, src):
                nc.scalar.dma_start(out=y[s * H:s * H + P0, :],
                                    in_=src[0:P0, 0:ROW])
                nc.scalar.dma_start(out=y[s * H + P0:s * H + H, :],
                                    in_=src[0:P1, ROW:2 * ROW])

            def rev_pixels(src, dst):
                for sl, n, eng in ((0, P0, "dve"), (1, P1, "act")):
                    sv = src[0:n, sl * ROW:sl * ROW + ROW].rearrange(
                        "p (j c) -> p j c", j=W, c=C)
                    dv = dst[0:n, sl * ROW:sl * ROW + ROW].rearrange(
                        "p (j c) -> p j c", j=W, c=C)
                    if eng == "dve":
                        nc.vector.tensor_copy(out=dv, in_=_flip(sv, 1))
                    else:
                        nc.scalar.copy(out=dv, in_=_flip(sv, 1))

            def transpose_pass(s, src, dst, mode, store=None, fine_last=False):
                # w outer so each output half completes (and can store)
                # while the other half is still being transposed
                for w, fw in ((0, P0), (1, P1)):  # dst row window
                    # partition split for the drain-exposed final window
                    fine = fine_last and w == 1
                    psplits = ((0, fw // 2), (fw // 2, fw)) if fine \
                        else ((0, fw),)
                    for sl, ps in ((0, P0), (1, P1)):   # source row slot
                        for g in range(32 // GRP):    # channel groups
                            pt = tpool.tile([128, 128 * GRP], BF16,
                                            name=f"pt{s}{g}{sl}{w}", tag="pt")
                            sv = _pixview(src[0:ps, sl * ROW:sl * ROW + ROW])
                            for cc in range(GRP):
                                ch = g * GRP + cc
                                nc.tensor.transpose(
                                    pt[0:fw, cc * 128:cc * 128 + ps],
                                    sv[0:ps, ch:ch + 1, w * 128:w * 128 + fw],
                                    it[0:ps, 0:ps])
                            dv = _pixview(dst[0:fw, w * ROW:w * ROW + ROW])
                            if mode == "T":
                                d3full = dv[0:fw, g * GRP:(g + 1) * GRP,
                                            sl * 128:sl * 128 + ps]
                            else:  # "k3": reversed pixel positions
                                j0 = 96 if sl == 0 else 0
                                d3full = _flip(
                                    dv[0:fw, g * GRP:(g + 1) * GRP,
                                       j0:j0 + ps], 2)
                            for f0, f1 in psplits:
                                d3 = bass.AP(
                                    d3full.tensor,
                                    d3full.offset + d3full.ap[0][0] * f0,
                                    [[d3full.ap[0][0], f1 - f0]]
                                    + [list(p) for p in d3full.ap[1:]])
                                src3 = bass.AP(
                                    pt[:].tensor,
                                    pt[:].offset + 128 * GRP * f0,
                                    [[128 * GRP, f1 - f0], [128, GRP],
                                     [1, ps]])
                                # DVE also carries the classify adds; ACT
                                # takes the larger share of copies
                                if g == 0:
                                    nc.vector.tensor_copy(out=d3, in_=src3)
                                else:
                                    nc.scalar.copy(out=d3, in_=src3)
                    if store is not None:
                        if fine:
                            for hi, (f0, f1) in enumerate(psplits):
                                store(w, f0, f1)
                        else:
                            store(w, 0, fw)

            for s in range(S):
                last = s == S - 1
                A = apool.tile([128, 2 * ROW], BF16, name=f"A{s}", tag="A")
                load(s, A)
                if with_cls:
                    sums = classify_sums(s, A)
                r = pattern[s]
                if r == 0:
                    store_fwd(s, A)
                    if with_cls:
                        classify_collapse(s, sums)
                    continue
                B = bpool.tile([128, 2 * ROW], BF16, name=f"B{s}", tag="B")
                if r == 1:
                    rev_pixels(A, B)
                    transpose_pass(s, B, A, "T")
                    store_fwd(s, A)
                elif r == 2:
                    rev_pixels(A, B)
                    transpose_pass(s, B, A, "T")
                    rev_pixels(A, B)
                    transpose_pass(s, B, A, "T")
                    store_fwd(s, A)
                else:  # r == 3
                    def store_w(w, f0, f1, s=s, B=B):
                        if w == 0:
                            nc.scalar.dma_start(
                                out=y[s * H + f0:s * H + f1, :],
                                in_=B[f0:f1, 0:ROW])
                        else:
                            nc.scalar.dma_start(
                                out=y[s * H + P0 + f0:s * H + P0 + f1, :],
                                in_=B[f0:f1, ROW:2 * ROW])
                    transpose_pass(s, A, B, "k3", store=store_w,
                                   fine_last=last)
                if with_cls:
                    classify_collapse(s, sums)
            if with_cls:
                nc.sync.dma_start(out=cs,
                                  in_=pmall[:].rearrange(
                                      "p (s c) -> (p s) c", s=S, c=C))
    nc.finalize()
    return nc


_NC_CACHE = {}


def get_rotate_nc(S, pattern):
    key = ("rot", S, pattern)
    if key not in _NC_CACHE:
        _NC_CACHE[key] = _build_rotate(S, pattern, with_cls=False)
    return _NC_CACHE[key]


def get_rotate_cls_nc(S, pattern):
    key = ("rotcls", S, pattern)
    if key not in _NC_CACHE:
        _NC_CACHE[key] = _build_rotate(S, pattern, with_cls=True)
    return _NC_CACHE[key]


def _ident_np():
    return np.eye(128, dtype=NP_BF16)


def run_rotate_cls(x_bf, pattern, W_cls, b_cls):
    """One launch: rotate per the speculated pattern AND emit per-channel
    sums; logits/argmax computed on host from the sums.

    x_bf: [B, H, W, C] bf16 (host-rounded).
    """
    B = x_bf.shape[0]
    S = B // N_CORES
    ident = _ident_np()
    in_maps = []
    for c in range(N_CORES):
        xs = np.ascontiguousarray(x_bf[c * S:(c + 1) * S].reshape(S * H, ROW))
        in_maps.append({"x": xs, "ident": ident})
    nc = get_rotate_cls_nc(S, pattern)
    res = run_bass_kernel_spmd(nc, in_maps, core_ids=list(range(N_CORES)))
    out = np.empty((B, H, W, C), dtype=np.float32)
    for c in range(N_CORES):
        out[c * S:(c + 1) * S] = np.asarray(
            res.results[c]["y"]).astype(np.float32).reshape(S, H, W, C)
    sums = np.concatenate(
        [np.asarray(res.results[c]["cs"]) for c in range(N_CORES)], axis=0)
    lg = (sums / float(H * W)) @ W_cls + b_cls
    return out, lg


def _np_fallback(x, W_cls, b_cls):
    mean = x.mean(axis=(1, 2))
    ks = np.argmax(mean @ W_cls + b_cls, axis=-1)
    out = np.empty_like(x)
    for i in range(x.shape[0]):
        out[i] = np.rot90(x[i], int(ks[i]), axes=(0, 1))
    return out


def kernel(x: np.ndarray, W_cls: np.ndarray, b_cls: np.ndarray) -> np.ndarray:
    x = np.asarray(x)
    B = x.shape[0]
    if x.shape != (B, H, W, C) or B % N_CORES != 0:
        return _np_fallback(np.asarray(x, dtype=np.float32),
                            np.asarray(W_cls, dtype=np.float32),
                            np.asarray(b_cls, dtype=np.float32))
    S = B // N_CORES
    x = np.ascontiguousarray(x, dtype=np.float32)
    W_cls = np.asarray(W_cls, dtype=np.float32)
    b_cls = np.asarray(b_cls, dtype=np.float32)
    x_bf = x.astype(NP_BF16)

    # Speculate the routing a priori: the classifier bias dominates the
    # near-zero image means, so argmax(b) predicts k for ~all samples.
    # The single launch rotates per the speculated pattern AND emits the
    # true per-channel sums; mispredicted samples are fixed up afterward.
    k_prior = int(np.argmax(b_cls))
    pattern = (k_prior,) * S
    out, lg = run_rotate_cls(x_bf, pattern, W_cls, b_cls)
    ks = np.argmax(lg, axis=-1).astype(np.int64)       # [B]

    bad = np.flatnonzero(ks != k_prior)
    if bad.size:
        # host fixup for mispredicted samples (rare: means would have to
        # overcome the bias gaps); f32-exact for these samples
        for b in bad:
            out[b] = np.rot90(x[b], int(ks[b]), axes=(0, 1))
    return out


# revision 8
# speedup vs baseline: 1.1738x; 1.1738x over previous
"""Trainium2 Bass kernel for DiscreteRotation (moe_routing).

Per sample: k = argmax(mean_hw(x) @ W + b); out = rot90(x, k, axes=(H,W)).

Pure data parallel over 8 NeuronCores (8 samples each). The rotation is a
pure permutation of the data, and the correctness gate is rel_err < 2e-2,
so the device pipeline runs in bfloat16: the host rounds x to bf16 (max
elementwise rel err 2^-9 ~ 0.2%), the device moves/permutes bf16 rows, and
the host upcasts the result. This halves HBM traffic per direction, which
is the roofline for this memory-bound problem (51.4 MB/core/direction in
f32 -> 25.7 MB in bf16 at 360 GB/s aggregate DMA).

Runtime branching is not supported by this execution path, so the routing
is SPECULATED: the classifier bias dominates the near-zero image means of
this input regime, so k_prior = argmax(b) predicts every sample's route. A
single static HW launch rotates every sample per the speculated pattern AND
computes per-sample per-channel pixel sums from the same SBUF-resident bf16
data (halving adds into a scratch tile + strided f32 reduce on DVE,
partition-collapse ones-matmul on PE after the transposes so it never
stalls them); the host turns the sums into logits/argmax and fixes up any
mispredicted sample with numpy rot90 on the original f32 data (correct for
arbitrary inputs, never triggered by the bias-dominated regime).

Queue discipline (each HWDGE queue drains in program order, so mixing
producers stalls the DMA bus): loads ride SP, stores ride Activation, the
identity load rides Pool/SWDGE, and the single channel-sums DMA fires once
at the end. The last sample's w=1 copies/stores are split by partition
halves to shorten the exposed pipeline drain.

Per-slot rotation code (all fully static):
  r=0: load -> store.
  r=1: load -> pixel-reverse within rows (DVE/ACT, negative-stride input
       APs) -> pixel-transpose on PE (per-channel [<=128 x <=128] tiles via
       identity matmul, PSUM -> DVE/ACT copy) -> store.
  r=2: rot90 applied twice.
  r=3: pixel-transpose, copy-out at reversed pixel positions, each output
       half stored as soon as its copies land -> store.

Every DMA descriptor is a contiguous image (half-)row on both HBM and SBUF
sides (>= 6 KB in bf16), so all transfers run at full HBM rate. SBUF row
layout: image row r lives at partition r%128, slot r//128 (slot0: rows
0..127, slot1: rows 128..223).
"""
import numpy as np
import ml_dtypes
from contextlib import ExitStack

import concourse.bass as bass
import concourse.bacc as bacc
import concourse.tile as tile
import concourse.mybir as mybir
from concourse.bass_utils import run_bass_kernel_spmd

F32 = mybir.dt.float32
BF16 = mybir.dt.bfloat16
NP_BF16 = ml_dtypes.bfloat16

N_CORES = 8
H = 224
W = 224
C = 32
ROW = W * C          # 7168 elems per image row (14336 B in bf16)
P0, P1 = 128, 96     # rows in slot 0 / slot 1
GRP = 8


def _flip(ap: bass.AP, dim: int) -> bass.AP:
    """Reverse iteration order of one AP dim."""
    pairs = [list(p) for p in ap.ap]
    stride, num = pairs[dim]
    off = ap.offset + stride * (num - 1)
    pairs[dim] = [-stride, num]
    return bass.AP(ap.tensor, off, pairs)


def _pixview(ap2d: bass.AP) -> bass.AP:
    """[p, ROW-range] -> [p, c(32), j(224)] channel-major pixel view."""
    return ap2d.rearrange("p (j c) -> p c j", j=W, c=C)


# load quarters: (slot, window, nrows, npix); window-major so the w=0
# transposes start as soon as their data lands
QUARTS = [(0, 0, P0, P0), (1, 0, P1, P0), (0, 1, P0, P1), (1, 1, P1, P1)]


def _build_rotate(S: int, pattern: tuple, with_cls: bool = True) -> bacc.Bacc:
    """Static program: sample slot s gets rotation pattern[s].

    with_cls: additionally compute per-sample per-channel pixel sums from
    the loaded data and write them to "cs" [S, C] — lets the host verify a
    speculated pattern without a separate pass over x.
    """
    nc = bacc.Bacc("TRN2", target_bir_lowering=False, debug=False,
                   num_devices=N_CORES)
    x = nc.dram_tensor("x", [S * H, ROW], BF16, kind="ExternalInput").ap()
    ident = nc.dram_tensor("ident", [128, 128], BF16,
                           kind="ExternalInput").ap()
    y = nc.dram_tensor("y", [S * H, ROW], BF16, kind="ExternalOutput").ap()
    if with_cls:
        cs = nc.dram_tensor("cs", [S, C], F32, kind="ExternalOutput").ap()

    with tile.TileContext(nc) as tc:
        with ExitStack() as ctx:
            cpool = ctx.enter_context(tc.tile_pool(name="consts", bufs=1))
            apool = ctx.enter_context(tc.tile_pool(name="A", bufs=3))
            bpool = ctx.enter_context(tc.tile_pool(name="B", bufs=2))
            tpool = ctx.enter_context(
                tc.tile_pool(name="ptrans", bufs=4, space="PSUM"))

            it = cpool.tile([128, 128], BF16)
            # Pool/SWDGE queue: keeps SP free for the first x loads
            nc.gpsimd.dma_start(out=it[:], in_=ident)
            if with_cls:
                spool = ctx.enter_context(tc.tile_pool(name="small", bufs=2))
                mpool = ctx.enter_context(
                    tc.tile_pool(name="pmean", bufs=2, space="PSUM"))
                ones = cpool.tile([128, 1], F32)
                pmall = cpool.tile([1, S * C], F32)
                nc.vector.memset(ones[:], 1.0)

            def classify_sums(s, A):
                """Per-channel pixel sums: halving adds (bf16, DVE 2x mode)
                into a scratch tile (A stays intact for the rotation), then
                a small strided f32 reduce. Returns the per-row sums tile;
                the partition collapse happens on PE after the transposes."""
                sc = spool.tile([128, 2 * 112 * C], BF16, name=f"sc{s}",
                                tag="sc")
                sums = spool.tile([128, 2 * C], F32, name=f"sm{s}", tag="sm")
                with nc.allow_low_precision(reason="bf16 mean-pool; bias "
                                            "gaps dwarf rounding noise"):
                    for sl, n in ((0, P0), (1, P1)):
                        base = sl * ROW
                        half = 112 * C
                        # first halving writes scratch, rest halve in place
                        nc.vector.tensor_add(
                            out=sc[0:n, sl * 112 * C:sl * 112 * C + half],
                            in0=A[0:n, base:base + half],
                            in1=A[0:n, base + half:base + 2 * half])
                        npix = 112
                        sbase = sl * 112 * C
                        while npix % 2 == 0 and npix > 14:
                            h2 = npix // 2 * C
                            nc.vector.tensor_add(
                                out=sc[0:n, sbase:sbase + h2],
                                in0=sc[0:n, sbase:sbase + h2],
                                in1=sc[0:n, sbase + h2:sbase + 2 * h2])
                            npix //= 2
                        av = sc[0:n, sbase:sbase + npix * C]
                        nc.vector.tensor_reduce(
                            out=sums[0:n, sl * C:(sl + 1) * C],
                            in_=bass.AP(av.tensor, av.offset,
                                        [list(av.ap[0]), [1, C], [C, npix]]),
                            axis=mybir.AxisListType.X, op=mybir.AluOpType.add)
                return sums

            def classify_collapse(s, sums):
                """Partition-collapse the per-row sums (PE, post-transposes)
                and park the result in the persistent pmall tile."""
                pm = mpool.tile([1, C], F32, name=f"pm{s}", tag="pm")
                nc.tensor.matmul(pm[0:1, 0:C], lhsT=ones[0:P0, 0:1],
                                 rhs=sums[0:P0, 0:C], start=True, stop=False)
                nc.tensor.matmul(pm[0:1, 0:C], lhsT=ones[0:P1, 0:1],
                                 rhs=sums[0:P1, C:2 * C], start=False,
                                 stop=True)
                # on ACT: its wait (pm ready) is already implied by PE
                # ordering, so it never head-of-line-blocks the next
                # sample's DVE work the way a DVE copy would
                nc.scalar.copy(out=pmall[0:1, s * C:(s + 1) * C],
                               in_=pm[0:1, 0:C])

            def load(s, A):
                for sl, w_, n, npx in QUARTS:
                    off = sl * ROW + w_ * 128 * C
                    xr = x[s * H + sl * 128:s * H + sl * 128 + n,
                           w_ * 128 * C:w_ * 128 * C + npx * C]
                    nc.sync.dma_start(out=A[0:n, off:off + npx * C], in_=xr)

            def store_fwd(s, src):
                nc.scalar.dma_start(out=y[s * H:s * H + P0, :],
                                    in_=src[0:P0, 0:ROW])
                nc.scalar.dma_start(out=y[s * H + P0:s * H + H, :],
                                    in_=src[0:P1, ROW:2 * ROW])

            def rev_pixels(src, dst):
                for sl, n, eng in ((0, P0, "dve"), (1, P1, "act")):
                    sv = src[0:n, sl * ROW:sl * ROW + ROW].rearrange(
                        "p (j c) -> p j c", j=W, c=C)
                    dv = dst[0:n, sl * ROW:sl * ROW + ROW].rearrange(
                        "p (j c) -> p j c", j=W, c=C)
                    if eng == "dve":
                        nc.vector.tensor_copy(out=dv, in_=_flip(sv, 1))
                    else:
                        nc.scalar.copy(out=dv, in_=_flip(sv, 1))

            def transpose_pass(s, src, dst, mode, store=None, fine_last=False):
                # w outer so each output half completes (and can store)
                # while the other half is still being transposed
                for w, fw in ((0, P0), (1, P1)):  # dst row window
                    # partition split for the drain-exposed final window
                    fine = fine_last and w == 1
                    psplits = ((0, fw // 2), (fw // 2, fw)) if fine \
                        else ((0, fw),)
                    for sl, ps in ((0, P0), (1, P1)):   # source row slot
                        for g in range(32 // GRP):    # channel groups
                            pt = tpool.tile([128, 128 * GRP], BF16,
                                            name=f"pt{s}{g}{sl}{w}", tag="pt")
                            sv = _pixview(src[0:ps, sl * ROW:sl * ROW + ROW])
                            for cc in range(GRP):
                                ch = g * GRP + cc
                                nc.tensor.transpose(
                                    pt[0:fw, cc * 128:cc * 128 + ps],
                                    sv[0:ps, ch:ch + 1, w * 128:w * 128 + fw],
                                    it[0:ps, 0:ps])
                            dv = _pixview(dst[0:fw, w * ROW:w * ROW + ROW])
                            if mode == "T":
                                d3full = dv[0:fw, g * GRP:(g + 1) * GRP,
                                            sl * 128:sl * 128 + ps]
                            else:  # "k3": reversed pixel positions
                                j0 = 96 if sl == 0 else 0
                                d3full = _flip(
                                    dv[0:fw, g * GRP:(g + 1) * GRP,
                                       j0:j0 + ps], 2)
                            for f0, f1 in psplits:
                                d3 = bass.AP(
                                    d3full.tensor,
                                    d3full.offset + d3full.ap[0][0] * f0,
                                    [[d3full.ap[0][0], f1 - f0]]
                                    + [list(p) for p in d3full.ap[1:]])
                                src3 = bass.AP(
                                    pt[:].tensor,
                                    pt[:].offset + 128 * GRP * f0,
                                    [[128 * GRP, f1 - f0], [128, GRP],
                                     [1, ps]])
                                # DVE also carries the classify adds; ACT
                                # takes the larger share of copies
                                if g == 0:
                                    nc.vector.tensor_copy(out=d3, in_=src3)
                                else:
                                    nc.scalar.copy(out=d3, in_=src3)
                    if store is not None:
                        if fine:
                            for hi, (f0, f1) in enumerate(psplits):
                                store(w, f0, f1)
                        else:
                            store(w, 0, fw)

            for s in range(S):
                last = s == S - 1
                A = apool.tile([128, 2 * ROW], BF16, name=f"A{s}", tag="A")
                load(s, A)
                if with_cls:
                    sums = classify_sums(s, A)
                r = pattern[s]
                if r == 0:
                    store_fwd(s, A)
                    if with_cls:
                        classify_collapse(s, sums)
                    continue
                B = bpool.tile([128, 2 * ROW], BF16, name=f"B{s}", tag="B")
                if r == 1:
                    rev_pixels(A, B)
                    transpose_pass(s, B, A, "T")
                    store_fwd(s, A)
                elif r == 2:
                    rev_pixels(A, B)
                    transpose_pass(s, B, A, "T")
                    rev_pixels(A, B)
                    transpose_pass(s, B, A, "T")
                    store_fwd(s, A)
                else:  # r == 3
                    def store_w(w, f0, f1, s=s, B=B):
                        if w == 0:
                            nc.scalar.dma_start(
                                out=y[s * H + f0:s * H + f1, :],
                                in_=B[f0:f1, 0:ROW])
                        else:
                            nc.scalar.dma_start(
                                out=y[s * H + P0 + f0:s * H + P0 + f1, :],
                                in_=B[f0:f1, ROW:2 * ROW])
                    transpose_pass(s, A, B, "k3", store=store_w,
                                   fine_last=last)
                if with_cls:
                    classify_collapse(s, sums)
            if with_cls:
                nc.sync.dma_start(out=cs,
                                  in_=pmall[:].rearrange(
                                      "p (s c) -> (p s) c", s=S, c=C))
    nc.finalize()
    return nc


_NC_CACHE = {}


def get_rotate_nc(S, pattern):
    key = ("rot", S, pattern)
    if key not in _NC_CACHE:
        _NC_CACHE[key] = _build_rotate(S, pattern, with_cls=False)
    return _NC_CACHE[key]


def get_rotate_cls_nc(S, pattern):
    key = ("rotcls", S, pattern)
    if key not in _NC_CACHE:
        _NC_CACHE[key] = _build_rotate(S, pattern, with_cls=True)
    return _NC_CACHE[key]


def _ident_np():
    return np.eye(128, dtype=NP_BF16)


def run_rotate_cls(x_bf, pattern, W_cls, b_cls):
    """One launch: rotate per the speculated pattern AND emit per-channel
    sums; logits/argmax computed on host from the sums.

    x_bf: [B, H, W, C] bf16 (host-rounded).
    """
    B = x_bf.shape[0]
    S = B // N_CORES
    ident = _ident_np()
    in_maps = []
    for c in range(N_CORES):
        xs = np.ascontiguousarray(x_bf[c * S:(c + 1) * S].reshape(S * H, ROW))
        in_maps.append({"x": xs, "ident": ident})
    nc = get_rotate_cls_nc(S, pattern)
    res = run_bass_kernel_spmd(nc, in_maps, core_ids=list(range(N_CORES)))
    out = np.empty((B, H, W, C), dtype=np.float32)
    for c in range(N_CORES):
        out[c * S:(c + 1) * S] = np.asarray(
            res.results[c]["y"]).astype(np.float32).reshape(S, H, W, C)
    sums = np.concatenate(
        [np.asarray(res.results[c]["cs"]) for c in range(N_CORES)], axis=0)
    lg = (sums / float(H * W)) @ W_cls + b_cls
    return out, lg


def _np_fallback(x, W_cls, b_cls):
    mean = x.mean(axis=(1, 2))
    ks = np.argmax(mean @ W_cls + b_cls, axis=-1)
    out = np.empty_like(x)
    for i in range(x.shape[0]):
        out[i] = np.rot90(x[i], int(ks[i]), axes=(0, 1))
    return out


def kernel(x: np.ndarray, W_cls: np.ndarray, b_cls: np.ndarray) -> np.ndarray:
    x = np.asarray(x)
    B = x.shape[0]
    if x.shape != (B, H, W, C) or B % N_CORES != 0:
        return _np_fallback(np.asarray(x, dtype=np.float32),
                            np.asarray(W_cls, dtype=np.float32),
                            np.asarray(b_cls, dtype=np.float32))
    S = B // N_CORES
    x = np.ascontiguousarray(x, dtype=np.float32)
    W_cls = np.asarray(W_cls, dtype=np.float32)
    b_cls = np.asarray(b_cls, dtype=np.float32)
    x_bf = x.astype(NP_BF16)

    # Speculate the routing a priori: the classifier bias dominates the
    # near-zero image means, so argmax(b) predicts k for ~all samples.
    # The single launch rotates per the speculated pattern AND emits the
    # true per-channel sums; mispredicted samples are fixed up afterward.
    k_prior = int(np.argmax(b_cls))
    pattern = (k_prior,) * S
    out, lg = run_rotate_cls(x_bf, pattern, W_cls, b_cls)
    ks = np.argmax(lg, axis=-1).astype(np.int64)       # [B]

    bad = np.flatnonzero(ks != k_prior)
    if bad.size:
        # host fixup for mispredicted samples (rare: means would have to
        # overcome the bias gaps); f32-exact for these samples
        for b in bad:
            out[b] = np.rot90(x[b], int(ks[b]), axes=(0, 1))
    return out
